# revision 30
# baseline (speedup 1.0000x reference)
"""Multi-head attention on 8 Trainium2 NeuronCores (Bass/Tile).

Sharding: batch B=4 x head-groups 2 -> 8 cores. Each core computes full
attention for 1 batch element and 8 of 16 heads, producing a partial
output projection (Wo row-sharded); host sums the two partials per batch.

Device dataflow (per core), everything in "transposed" orientation so the
contraction dim always sits on SBUF partitions. All matmul operands are
bf16 (fp32 PSUM accumulation).
  qT/kT [128, quarter, c, 512] bf16 quarter-major so early S-quarters land
  first; vT [128, c, S] bf16. All activations DMA'd ONCE and stay resident
  in SBUF (no re-loads; trace showed 24MB of repeat DMA stalling stage 1).
  V proj c-outer/st-inner across 8 PSUM banks: matmuls start as soon as
  vT chunk c lands instead of waiting for the full tensor. bv is folded
  into the DVE PSUM->SBUF copy (tensor_tensor add with a pre-broadcast
  bias tile) instead of a ones-row matmul.
  QT/KT t=0 projected up front; t=1..3 interleaved one matmul per
  attention column (chains) to fill the PE while ACT paces the softmax.
  scores^T[k,q] = (K_h^T tile).T @ Q_h^T, N=512 per head (even/odd heads
  on partitions 0-63/64-127).
  expS: ACT table Exp for most k-tiles; for kt in SCHRAUDOLPH_KT the exp
  runs on DVE as a Schraudolph tensor_scalar (x*C1+C2 -> int16, bitcast
  bf16) so the softmax is not single-engine-paced. PV matmuls are skewed
  two columns behind scores/exp so the PE never waits the ACT latency.
  PV: out'[65, q] += Vp_tile.T @ expS_tile accumulated over 16 k-tiles;
  row 64 is the softmax denominator (ones column trick).
  A^T = out'[0:64] * reciprocal(out'[64]) broadcast -> bf16 [512, S].
  O^T[m, s] = Wo_chunk.T @ A^T chunk accumulated over 4 chunks -> bf16.
  The output projection is fully interleaved into the last attention
  columns (2 matmuls/col from col ~209) - no separate stage 3.
Host: out[b] = (O^T_hg0 + O^T_hg1).T + bo.
"""

import sys

sys.path.insert(0, "/opt/trn_rl_repo")

import ml_dtypes
import numpy as np

import concourse.bacc as bacc
import concourse.mybir as mybir
from concourse import tile
from concourse.bass_utils import run_bass_kernel_spmd

F32 = mybir.dt.float32
BF16 = mybir.dt.bfloat16
I16 = mybir.dt.int16
AF = mybir.ActivationFunctionType
ALU = mybir.AluOpType
NP_BF16 = ml_dtypes.bfloat16

H, DK, DV, DM = 16, 64, 64, 1024
B, S = 4, 2048
HL = H // 2          # heads per core
NB = HL * DK         # 512: per-core projection width
NDM = DM // 128      # 8 contraction chunks
NT = NB // 128       # 4 row-tiles of QT/KT/AT
NKT = S // 128       # 16 k-tiles
NQ = S // 512        # 4 s-quarters
SCALE = 1.0 / 8.0    # 1/sqrt(DK)
# Schraudolph approximate exp: exp(s*SCALE) ~ bitcast_bf16(int16(s*C1+C2)).
# +0.5 biases the float->int16 truncation to round-to-nearest; if hw
# rounds anyway it is a constant factor on every weight and cancels in
# the softmax normalization.
SC_C1 = SCALE * 128.0 * 1.4426950408889634
SC_C2 = 128.0 * (127.0 - 0.0436) + 0.5
# k-tiles whose exp runs on DVE (Schraudolph) instead of ACT
SCHRAUDOLPH_KT = (5, 10)

_CACHED_NC = None


def _build():
    nc = bacc.Bacc("TRN2", debug=False)

    qT = nc.dram_tensor("qT", [128, NQ, NDM, 512], BF16, kind="ExternalInput")
    kT = nc.dram_tensor("kT", [128, NQ, NDM, 512], BF16, kind="ExternalInput")
    vT = nc.dram_tensor("vT", [128, NDM, S], BF16, kind="ExternalInput")
    wq = nc.dram_tensor("wq", [128, NDM, NB], BF16, kind="ExternalInput")
    wk = nc.dram_tensor("wk", [128, NDM, NB], BF16, kind="ExternalInput")
    wv = nc.dram_tensor("wv", [128, NDM, NB], BF16, kind="ExternalInput")
    wo = nc.dram_tensor("wo", [128, NT, DM], BF16, kind="ExternalInput")
    bq = nc.dram_tensor("bq", [NB], F32, kind="ExternalInput")
    bk = nc.dram_tensor("bk", [NB], F32, kind="ExternalInput")
    # bv pre-broadcast to 128 partitions on the HOST: a device-side
    # partition_broadcast on the gpsimd ring blocks that ring's DMA
    # triggers behind a ucode lib load + sem wait for ~15us.
    bvb_in = nc.dram_tensor("bvb", [128, HL, DV], BF16, kind="ExternalInput")
    outT = nc.dram_tensor("outT", [DM, S], BF16, kind="ExternalOutput")

    with tile.TileContext(nc) as tc:
        with tc.tile_pool(name="persist", bufs=1) as persist:
            QT = persist.tile([128, NT, S], BF16)
            KT = persist.tile([128, NT, S], BF16)
            Vp = persist.tile([128, NKT, HL, DV + 1], BF16)
            wo_sb = persist.tile([128, NT, DM], BF16)
            bq_sb = persist.tile([128, NT], F32)
            bk_sb = persist.tile([128, NT], F32)
            bvb = persist.tile([128, HL, DV], BF16)

            nc.vector.memset(Vp[:, :, :, DV : DV + 1], 1.0)

            wgt_pool = tc.alloc_tile_pool(name="wgt", bufs=1)
            actqk_pool = tc.alloc_tile_pool(name="actqk", bufs=1)
            actv_pool = tc.alloc_tile_pool(name="actv", bufs=1)

            wts = {
                k: wgt_pool.tile([128, NDM, NB], BF16, tag=f"w{k}", name=f"w{k}")
                for k in ("q", "k", "v")
            }
            acts = {
                "q": actqk_pool.tile([128, NQ, NDM, 512], BF16, tag="aq", name="aq"),
                "k": actqk_pool.tile([128, NQ, NDM, 512], BF16, tag="ak", name="ak"),
                "v": actv_pool.tile([128, NDM, S], BF16, tag="av", name="av"),
            }

            # ---- All input DMAs up front, priority order, 2 rings ----
            # ring A = sync, ring B = gpsimd. wv/bv first (V proj is the
            # critical path), vT chunks alternating, then q, then k, wo last.
            nc.sync.dma_start(wts["v"][:, 0:4, :], wv[:, 0:4, :])
            nc.gpsimd.dma_start(wts["v"][:, 4:8, :], wv[:, 4:8, :])
            nc.gpsimd.dma_start(bvb[:], bvb_in[:])
            for c in range(NDM):
                eng = nc.sync if c % 2 == 0 else nc.gpsimd
                eng.dma_start(acts["v"][:, c, :], vT[:, c, :])
            nc.sync.dma_start(wts["q"][:, 0:4, :], wq[:, 0:4, :])
            nc.gpsimd.dma_start(wts["q"][:, 4:8, :], wq[:, 4:8, :])
            nc.sync.dma_start(bq_sb[:], bq.rearrange("(t p) -> p t", p=128))
            nc.sync.dma_start(bk_sb[:], bk.rearrange("(t p) -> p t", p=128))
            nc.sync.dma_start(wts["k"][:, 0:4, :], wk[:, 0:4, :])
            nc.gpsimd.dma_start(wts["k"][:, 4:8, :], wk[:, 4:8, :])
            # Interleave q/k quarters so the K-t0 projections (emitted right
            # after Q-t0) are not starved by a back-loaded kT transfer.
            for q in range(NQ):
                nc.sync.dma_start(acts["q"][:, q, :, :], qT[:, q, :, :])
                nc.gpsimd.dma_start(acts["k"][:, q, :, :], kT[:, q, :, :])
            nc.gpsimd.dma_start(wo_sb[:], wo[:])
            # Pre-warm the gpsimd broadcast ucode lib during stage 1: the
            # first partition_broadcast pays ~7us of lib load + ring fence,
            # which would otherwise stall the first norm (and through the
            # PV-slot rotation, the PE) in early stage 2.
            warm_bc = persist.tile([2, 8], BF16, name="warmbc")
            nc.gpsimd.partition_broadcast(warm_bc[:], bvb[0:1, 0, 0:8])

            ps_s1 = tc.alloc_tile_pool(name="ps_s1", bufs=8, space="PSUM")

            # PE warm-up while the DMA head streams vT: matmuls on a memset
            # scratch tile keep the HAM clock at 8/8. The warm PSUM buffer
            # is one slot of the "ps" rotation; it is reused as a V-proj
            # bank only after the warm matmuls (first in PE order) retire.
            warm_src = persist.tile([64, 128], BF16, name="warmsrc")
            nc.vector.memset(warm_src[:], 0.0)
            wps = ps_s1.tile([128, 512], F32, tag="ps", name="warmps")

            def warm_mm(n=1):
                for _ in range(n):
                    nc.tensor.matmul(
                        wps[:, 0:64],
                        warm_src[:, 0:128],
                        warm_src[:, 0:64],
                        start=True,
                        stop=True,
                    )

            warm_mm(150)

            # ---- V projection ----
            # First half (st 0-7): c-outer / st-inner over 8 PSUM banks so
            # matmuls for chunk c start as soon as vT[:, c, :] lands.
            # Second half (st 8-15): all data is resident by then, so go
            # st-outer — one bank at a time — which spreads the DVE drain
            # copies instead of bunching all 8 at the end (that bunching
            # stalled the next PSUM allocation for ~6us).
            # bv is added during the DVE copy.
            vps = [
                ps_s1.tile([128, 512], F32, tag="ps", name=f"vpsA{i}")
                for i in range(8)
            ]
            for c in range(NDM):
                for i in range(8):
                    nc.tensor.matmul(
                        vps[i][:],
                        acts["v"][:, c, i * 128 : (i + 1) * 128],
                        wts["v"][:, c, :],
                        start=(c == 0),
                        stop=(c == NDM - 1),
                    )

            def vp_drain(st, ps):
                nc.vector.tensor_tensor(
                    Vp[:, st, :, 0:DV],
                    ps[:].rearrange("p (h d) -> p h d", h=HL),
                    bvb[:],
                    op=ALU.add,
                )

            for i in range(8):
                vp_drain(i, vps[i])
            for i in range(8):
                st = 8 + i
                psB = ps_s1.tile([128, 512], F32, tag="ps", name=f"vpsB{i}")
                for c in range(NDM):
                    nc.tensor.matmul(
                        psB[:],
                        acts["v"][:, c, st * 128 : (st + 1) * 128],
                        wts["v"][:, c, :],
                        start=(c == 0),
                        stop=(c == NDM - 1),
                    )
                vp_drain(st, psB)

            actv_pool.release()

            # ---- Q/K projections for t=0 (quarter-granular) ----
            biases = {"q": bq_sb, "k": bk_sb}
            dsts = {"q": QT, "k": KT}
            for key in ("q", "k"):
                for quarter in range(NQ):
                    qs = slice(quarter * 512, quarter * 512 + 512)
                    ps = ps_s1.tile([128, 512], F32, tag="ps")
                    for c in range(NDM):
                        nc.tensor.matmul(
                            ps[:],
                            wts[key][:, c, 0:128],
                            acts[key][:, quarter, c, :],
                            start=(c == 0),
                            stop=(c == NDM - 1),
                        )
                    nc.vector.tensor_scalar_add(
                        dsts[key][:, 0, qs], ps[:], biases[key][:, 0:1]
                    )
            ps_s1.release()

            # ---- Stage 2: attention with interleaved projections ----
            with tc.tile_pool(name="att", bufs=1) as att_pool:
                AT = att_pool.tile([128, NT, S], BF16)
                with (
                    tc.tile_pool(name="expS", bufs=6) as exp_pool,
                    tc.tile_pool(name="rec", bufs=3) as rec_pool,
                    tc.tile_pool(name="ost2", bufs=3) as ost2_pool,
                    tc.tile_pool(name="ps_sc", bufs=2, space="PSUM") as ps_sc,
                    tc.tile_pool(name="ps_pv", bufs=4, space="PSUM") as ps_pv,
                ):
                    # Flat column stream over (hp, qb, kt) with the PV
                    # matmuls skewed one column behind scores/exp.
                    pv_store = {}
                    proj_fns = {}
                    for hp in range(HL // 2):
                        chains = []
                        if hp < NT - 1:
                            tn = hp + 1
                            # K chains first: KT[tn] must cover full S before
                            # the hp=tn columns start; QT[tn] only needs to
                            # stay ahead of that block's qb loop.
                            for key in ("k", "q"):
                                for quarter in range(NQ):
                                    chains.append((key, tn, quarter))
                        chain_ps = [None]

                        def make_proj_tick(chains, chain_ps):
                            def proj_tick(tick):
                                ci, step = tick // 8, tick % 8
                                if ci >= len(chains):
                                    return
                                key, tn, quarter = chains[ci]
                                if step == 0:
                                    chain_ps[0] = ps_pv.tile(
                                        [128, 512], F32, tag="pv", name="projps"
                                    )
                                cps = chain_ps[0]
                                nc.tensor.matmul(
                                    cps[:],
                                    wts[key][:, step, tn * 128 : (tn + 1) * 128],
                                    acts[key][:, quarter, step, :],
                                    start=(step == 0),
                                    stop=(step == NDM - 1),
                                )
                                if step == NDM - 1:
                                    nc.vector.tensor_scalar_add(
                                        dsts[key][
                                            :,
                                            tn,
                                            quarter * 512 : quarter * 512 + 512,
                                        ],
                                        cps[:],
                                        biases[key][:, tn : tn + 1],
                                    )

                            return proj_tick

                        proj_fns[hp] = make_proj_tick(chains, chain_ps)

                    cols = [
                        (hp, qb, kt)
                        for hp in range(HL // 2)
                        for qb in range(4)
                        for kt in range(NKT)
                    ]

                    def emit_pv(hp, qb, kt, ex):
                        pvs = pv_store[(hp, qb)]
                        for sub in range(2):
                            nc.tensor.matmul(
                                pvs[sub][0 : DV + 1, :],
                                Vp[:, kt, hp * 2 + sub, :],
                                ex[:, sub, :],
                                start=(kt == 0),
                                stop=(kt == NKT - 1),
                            )

                    def emit_norm(hp, qb):
                        t = hp
                        qsl = slice(qb * 512, (qb + 1) * 512)
                        pvs = pv_store.pop((hp, qb))
                        work = []
                        # Drain PSUM to SBUF first so the PV banks free after
                        # ~0.6us of copies instead of after the full
                        # recip -> gpsimd broadcast -> mul chain (~1.6us);
                        # the next group's first PV start was stalling on it.
                        for sub in range(2):
                            pvp = pvs[sub]
                            dcp = rec_pool.tile([1, 512], F32, tag="d")
                            pvc = rec_pool.tile([64, 512], F32, tag="pc")
                            # custom-DVE ucode mishandles base_partition=64
                            # PSUM reads; stage through partition 0
                            nc.vector.tensor_copy(dcp[:], pvp[DV : DV + 1, :])
                            nc.vector.tensor_copy(pvc[:], pvp[0:DV, :])
                            work.append((dcp, pvc))
                        for sub in range(2):
                            psl = slice(sub * 64, sub * 64 + 64)
                            dcp, pvc = work[sub]
                            rec = rec_pool.tile([1, 512], F32, tag="r")
                            recb = rec_pool.tile([64, 512], F32, tag="rb")
                            nc.vector.reciprocal_approx_fast(rec[:], dcp[:])
                            nc.gpsimd.partition_broadcast(recb[:], rec[:])
                            nc.vector.tensor_mul(
                                AT[psl, t, qsl], pvc[:], recb[:]
                            )

                    # Output projection fully interleaved into the last
                    # attention columns. Job (m, qh) needs AT[:, :, qh*512:]
                    # complete, i.e. norm(hp=3, qb=qh) at col ~208+16*qh.
                    oproj_jobs = [(m, qh) for qh in range(4) for m in range(NDM)]
                    oproj_state = {"i": 0, "ps": None}

                    def oproj_tick():
                        i = oproj_state["i"]
                        ci, step = i // NT, i % NT
                        if ci >= len(oproj_jobs):
                            return False
                        m, qh = oproj_jobs[ci]
                        hs = slice(qh * 512, qh * 512 + 512)
                        if step == 0:
                            oproj_state["ps"] = ps_pv.tile(
                                [128, 512], F32, tag="pv", name="ops"
                            )
                        ps = oproj_state["ps"]
                        nc.tensor.matmul(
                            ps[:],
                            wo_sb[:, step, m * 128 : (m + 1) * 128],
                            AT[:, step, hs],
                            start=(step == 0),
                            stop=(step == NT - 1),
                        )
                        if step == NT - 1:
                            ot = ost2_pool.tile([128, 512], BF16, tag="os")
                            nc.vector.tensor_copy(ot[:], ps[:])
                            nc.sync.dma_start(
                                outT[m * 128 : (m + 1) * 128, hs], ot[:]
                            )
                        oproj_state["i"] = i + 1
                        return True

                    # PV matmuls are skewed TWO columns behind scores/exp so
                    # the PE never catches the ACT latency tail (a 1-column
                    # skew left ~300-400ns PV-waits-exp stalls at every
                    # group boundary).
                    pipe = []
                    for hp, qb, kt in cols:
                        t = hp
                        qsl = slice(qb * 512, (qb + 1) * 512)
                        if (hp, qb) not in pv_store:
                            pv_store[(hp, qb)] = [
                                ps_pv.tile([128, 512], F32, tag="pv", name=f"pv{i}")
                                for i in range(2)
                            ]
                        scp = ps_sc.tile([128, 2, 512], F32, tag="sc")
                        for sub in range(2):
                            psl = slice(sub * 64, sub * 64 + 64)
                            nc.tensor.matmul(
                                scp[:, sub, :],
                                KT[psl, t, kt * 128 : (kt + 1) * 128],
                                QT[psl, t, qsl],
                                start=True,
                                stop=True,
                            )
                        ex = exp_pool.tile([128, 2, 512], BF16, tag="e")
                        if kt in SCHRAUDOLPH_KT:
                            nc.vector.tensor_scalar(
                                ex[:].bitcast(I16),
                                scp[:],
                                SC_C1,
                                SC_C2,
                                op0=ALU.mult,
                                op1=ALU.add,
                            )
                        else:
                            nc.scalar.activation(ex[:], scp[:], AF.Exp, scale=SCALE)
                        if len(pipe) >= 2:
                            phps, pqb, pkt, pex = pipe.pop(0)
                            emit_pv(phps, pqb, pkt, pex)
                            if pkt == NKT - 1:
                                emit_norm(phps, pqb)
                        proj_fns[hp](qb * NKT + kt)
                        ci_flat = (hp * 4 + qb) * NKT + kt
                        if ci_flat >= 210:
                            oproj_tick()
                            oproj_tick()
                        pipe.append((hp, qb, kt, ex))
                    for phps, pqb, pkt, pex in pipe:
                        emit_pv(phps, pqb, pkt, pex)
                        if pkt == NKT - 1:
                            emit_norm(phps, pqb)
                    # Drain the remaining output-projection jobs in batches
                    # of 4 with steps 0..2 first (they only need AT rows
                    # 0..2, done long ago) so they overlap the final norm's
                    # DVE latency; only the step-3 matmuls wait on it.
                    # Interleaving across jobs also alternates PSUM banks.
                    rem = oproj_state["i"] // NT
                    assert oproj_state["i"] % NT == 0
                    pending = oproj_jobs[rem:]
                    for bstart in range(0, len(pending), 2):
                        batch = pending[bstart : bstart + 2]
                        bps = [
                            ps_pv.tile([128, 512], F32, tag="pv", name="dps")
                            for _ in batch
                        ]
                        for step in range(NT):
                            for (m, qh), ps in zip(batch, bps):
                                hs = slice(qh * 512, qh * 512 + 512)
                                nc.tensor.matmul(
                                    ps[:],
                                    wo_sb[:, step, m * 128 : (m + 1) * 128],
                                    AT[:, step, hs],
                                    start=(step == 0),
                                    stop=(step == NT - 1),
                                )
                        for (m, qh), ps in zip(batch, bps):
                            hs = slice(qh * 512, qh * 512 + 512)
                            ot = ost2_pool.tile([128, 512], BF16, tag="os")
                            nc.vector.tensor_copy(ot[:], ps[:])
                            nc.sync.dma_start(
                                outT[m * 128 : (m + 1) * 128, hs], ot[:]
                            )
            actqk_pool.release()
            wgt_pool.release()

    nc.compile()
    return nc


def get_nc():
    global _CACHED_NC
    if _CACHED_NC is None:
        _CACHED_NC = _build()
    return _CACHED_NC


def _bf(x):
    return np.ascontiguousarray(np.asarray(x, np.float32)).astype(NP_BF16)


def _tile_rows(x):
    # [R, C] -> [128, R//128, C] so each SBUF partition's data is contiguous
    r, c = x.shape
    return np.ascontiguousarray(x.reshape(r // 128, 128, c).transpose(1, 0, 2))


def _quarter_major(x):
    # [128, NDM, S] -> [128, NQ, NDM, 512]: s-quarter-major so early query
    # ranges finish DMA first
    t = x.reshape(128, NDM, NQ, 512)
    return np.ascontiguousarray(t.transpose(0, 2, 1, 3))


def make_in_maps(queries, keys, values, Wq, bq, Wk, bk, Wv, bv, Wo, bo):
    queries = np.asarray(queries, np.float32)
    keys = np.asarray(keys, np.float32)
    values = np.asarray(values, np.float32)
    Wq = np.asarray(Wq, np.float32)
    Wk = np.asarray(Wk, np.float32)
    Wv = np.asarray(Wv, np.float32)
    Wo = np.asarray(Wo, np.float32)
    bq = np.asarray(bq, np.float32)
    bk = np.asarray(bk, np.float32)
    bv = np.asarray(bv, np.float32)
    in_maps = []
    for core in range(8):
        b, hg = divmod(core, 2)
        sl = slice(hg * NB, (hg + 1) * NB)
        in_maps.append(
            {
                "qT": _quarter_major(_tile_rows(_bf(queries[b].T))),
                "kT": _quarter_major(_tile_rows(_bf(keys[b].T))),
                "vT": _tile_rows(_bf(values[b].T)),
                "wq": _tile_rows(_bf(Wq[:, sl])),
                "wk": _tile_rows(_bf(Wk[:, sl])),
                "wv": _tile_rows(_bf(Wv[:, sl])),
                "wo": _tile_rows(_bf(Wo[sl, :])),
                "bq": np.ascontiguousarray(bq[sl]),
                "bk": np.ascontiguousarray(bk[sl]),
                "bvb": np.ascontiguousarray(
                    np.broadcast_to(
                        _bf(bv[sl]).reshape(1, HL, DV), (128, HL, DV)
                    )
                ),
            }
        )
    return in_maps


def assemble(results, bo):
    bo = np.asarray(bo, np.float32)
    out = np.empty((B, S, DM), np.float32)
    for b in range(B):
        acc = np.asarray(results[2 * b]["outT"], np.float32) + np.asarray(
            results[2 * b + 1]["outT"], np.float32
        )
        out[b] = acc.T + bo
    return out


def run(trace=False, **inputs):
    if trace:
        # NTFF profiling shim: this image's antenv lacks axon_hooks.
        import types

        try:
            from antenv import axon_hooks  # noqa: F401
        except ImportError:
            from trn_agent_boot.trn_boot import _ntff_profile_via_ctypes

            mod = types.ModuleType("antenv.axon_hooks")
            _hook = _ntff_profile_via_ctypes("/opt/axon/libaxon_pjrt.so")
            mod.get_axon_ntff_profile_hook = lambda: _hook
            sys.modules["antenv.axon_hooks"] = mod
    nc = get_nc()
    bo = inputs["bo"]
    in_maps = make_in_maps(**inputs)
    res = run_bass_kernel_spmd(nc, in_maps, list(range(8)), trace=trace)
    return assemble(res.results, bo), res


def kernel(**inputs):
    out, _ = run(trace=False, **inputs)
    return out


# revision 32
# speedup vs baseline: 1.2148x; 1.2148x over previous
"""Multi-head attention on 8 Trainium2 NeuronCores (Bass/Tile).

Sharding: batch B=4 x head-groups 2 -> 8 cores. Each core computes full
attention for 1 batch element and 8 of 16 heads, producing a partial
output projection (Wo row-sharded); host sums the two partials per batch.

Device dataflow (per core), everything in "transposed" orientation so the
contraction dim always sits on SBUF partitions. All matmul operands are
bf16 (fp32 PSUM accumulation).
  qT/kT [128, quarter, c, 512] bf16 quarter-major so early S-quarters land
  first; vT [128, c, S] bf16. All activations DMA'd ONCE and stay resident
  in SBUF (no re-loads; trace showed 24MB of repeat DMA stalling stage 1).
  V proj c-outer/st-inner across 8 PSUM banks: matmuls start as soon as
  vT chunk c lands instead of waiting for the full tensor. bv is folded
  into the DVE PSUM->SBUF copy (tensor_tensor add with a pre-broadcast
  bias tile) instead of a ones-row matmul.
  QT/KT t=0 projected up front; t=1..3 interleaved one matmul per
  attention column (chains) to fill the PE while ACT paces the softmax.
  scores^T[k,q] = (K_h^T tile).T @ Q_h^T, N=512 per head (even/odd heads
  on partitions 0-63/64-127).
  expS: ACT table Exp for most k-tiles; for kt in SCHRAUDOLPH_KT the exp
  runs on DVE as a Schraudolph tensor_scalar (x*C1+C2 -> int16, bitcast
  bf16) so the softmax is not single-engine-paced. PV matmuls are skewed
  two columns behind scores/exp so the PE never waits the ACT latency.
  PV: out'[65, q] += Vp_tile.T @ expS_tile accumulated over 16 k-tiles;
  row 64 is the softmax denominator (ones column trick).
  A^T = out'[0:64] * reciprocal(out'[64]) broadcast -> bf16 [512, S].
  O^T[m, s] = Wo_chunk.T @ A^T chunk accumulated over 4 chunks -> bf16.
  The output projection is fully interleaved into the last attention
  columns (2 matmuls/col from col ~209) - no separate stage 3.
Host: out[b] = (O^T_hg0 + O^T_hg1).T + bo.
"""

import sys

sys.path.insert(0, "/opt/trn_rl_repo")

import ml_dtypes
import numpy as np

import concourse.bacc as bacc
import concourse.mybir as mybir
from concourse import tile
from concourse.bass_utils import run_bass_kernel_spmd

F32 = mybir.dt.float32
BF16 = mybir.dt.bfloat16
I16 = mybir.dt.int16
AF = mybir.ActivationFunctionType
ALU = mybir.AluOpType
NP_BF16 = ml_dtypes.bfloat16

H, DK, DV, DM = 16, 64, 64, 1024
B, S = 4, 2048
HL = H // 2          # heads per core
NB = HL * DK         # 512: per-core projection width
NDM = DM // 128      # 8 contraction chunks
NT = NB // 128       # 4 row-tiles of QT/KT/AT
NKT = S // 128       # 16 k-tiles
NQ = S // 512        # 4 s-quarters
SCALE = 1.0 / 8.0    # 1/sqrt(DK)
# Schraudolph approximate exp: exp(s*SCALE) ~ bitcast_bf16(int16(s*C1+C2)).
# +0.5 biases the float->int16 truncation to round-to-nearest; if hw
# rounds anyway it is a constant factor on every weight and cancels in
# the softmax normalization.
SC_C1 = SCALE * 128.0 * 1.4426950408889634
SC_C2 = 128.0 * (127.0 - 0.0436) + 0.5
# k-tiles whose exp runs on DVE (Schraudolph) instead of ACT
SCHRAUDOLPH_KT = (5, 10)

_CACHED_NC = None


def _build():
    nc = bacc.Bacc("TRN2", debug=False)

    qT = nc.dram_tensor("qT", [128, NQ, NDM, 512], BF16, kind="ExternalInput")
    kT = nc.dram_tensor("kT", [128, NQ, NDM, 512], BF16, kind="ExternalInput")
    vT = nc.dram_tensor("vT", [128, NDM, S], BF16, kind="ExternalInput")
    wq = nc.dram_tensor("wq", [128, NDM, NB], BF16, kind="ExternalInput")
    wk = nc.dram_tensor("wk", [128, NDM, NB], BF16, kind="ExternalInput")
    wv = nc.dram_tensor("wv", [128, NDM, NB], BF16, kind="ExternalInput")
    wo = nc.dram_tensor("wo", [128, NT, DM], BF16, kind="ExternalInput")
    bq = nc.dram_tensor("bq", [NB], F32, kind="ExternalInput")
    bk = nc.dram_tensor("bk", [NB], F32, kind="ExternalInput")
    # bv pre-broadcast to 128 partitions on the HOST: a device-side
    # partition_broadcast on the gpsimd ring blocks that ring's DMA
    # triggers behind a ucode lib load + sem wait for ~15us.
    bvb_in = nc.dram_tensor("bvb", [128, HL, DV], BF16, kind="ExternalInput")
    outT = nc.dram_tensor("outT", [DM, S], BF16, kind="ExternalOutput")

    with tile.TileContext(nc) as tc:
        with tc.tile_pool(name="persist", bufs=1) as persist:
            QT = persist.tile([128, NT, S], BF16)
            KT = persist.tile([128, NT, S], BF16)
            Vp = persist.tile([128, NKT, HL, DV + 1], BF16)
            wo_sb = persist.tile([128, NT, DM], BF16)
            bq_sb = persist.tile([128, NT], F32)
            bk_sb = persist.tile([128, NT], F32)
            bvb = persist.tile([128, HL, DV], BF16)

            nc.vector.memset(Vp[:, :, :, DV : DV + 1], 1.0)

            wgt_pool = tc.alloc_tile_pool(name="wgt", bufs=1)
            actqk_pool = tc.alloc_tile_pool(name="actqk", bufs=1)
            actv_pool = tc.alloc_tile_pool(name="actv", bufs=1)

            wts = {
                k: wgt_pool.tile([128, NDM, NB], BF16, tag=f"w{k}", name=f"w{k}")
                for k in ("q", "k", "v")
            }
            acts = {
                "q": actqk_pool.tile([128, NQ, NDM, 512], BF16, tag="aq", name="aq"),
                "k": actqk_pool.tile([128, NQ, NDM, 512], BF16, tag="ak", name="ak"),
                "v": actv_pool.tile([128, NDM, S], BF16, tag="av", name="av"),
            }

            # ---- All input DMAs up front, priority order, 2 rings ----
            # ring A = sync, ring B = gpsimd. wv/bv first (V proj is the
            # critical path), vT chunks alternating, then q, then k, wo last.
            nc.sync.dma_start(wts["v"][:, 0:4, :], wv[:, 0:4, :])
            nc.gpsimd.dma_start(wts["v"][:, 4:8, :], wv[:, 4:8, :])
            nc.gpsimd.dma_start(bvb[:], bvb_in[:])
            for c in range(NDM):
                eng = nc.sync if c % 2 == 0 else nc.gpsimd
                eng.dma_start(acts["v"][:, c, :], vT[:, c, :])
            nc.sync.dma_start(wts["q"][:, 0:4, :], wq[:, 0:4, :])
            nc.gpsimd.dma_start(wts["q"][:, 4:8, :], wq[:, 4:8, :])
            nc.sync.dma_start(bq_sb[:], bq.rearrange("(t p) -> p t", p=128))
            nc.sync.dma_start(bk_sb[:], bk.rearrange("(t p) -> p t", p=128))
            nc.sync.dma_start(wts["k"][:, 0:4, :], wk[:, 0:4, :])
            nc.gpsimd.dma_start(wts["k"][:, 4:8, :], wk[:, 4:8, :])
            # Interleave q/k quarters so the K-t0 projections (emitted right
            # after Q-t0) are not starved by a back-loaded kT transfer.
            for q in range(NQ):
                nc.sync.dma_start(acts["q"][:, q, :, :], qT[:, q, :, :])
                nc.gpsimd.dma_start(acts["k"][:, q, :, :], kT[:, q, :, :])
            nc.gpsimd.dma_start(wo_sb[:], wo[:])
            # Pre-warm the gpsimd broadcast ucode lib during stage 1: the
            # first partition_broadcast pays ~7us of lib load + ring fence,
            # which would otherwise stall the first norm (and through the
            # PV-slot rotation, the PE) in early stage 2.
            warm_bc = persist.tile([2, 8], BF16, name="warmbc")
            nc.gpsimd.partition_broadcast(warm_bc[:], bvb[0:1, 0, 0:8])

            ps_s1 = tc.alloc_tile_pool(name="ps_s1", bufs=8, space="PSUM")

            # PE warm-up while the DMA head streams vT: matmuls on a memset
            # scratch tile keep the HAM clock at 8/8. The warm PSUM buffer
            # is one slot of the "ps" rotation; it is reused as a V-proj
            # bank only after the warm matmuls (first in PE order) retire.
            warm_src = persist.tile([64, 128], BF16, name="warmsrc")
            nc.vector.memset(warm_src[:], 0.0)
            wps = ps_s1.tile([128, 512], F32, tag="ps", name="warmps")

            def warm_mm(n=1):
                for _ in range(n):
                    nc.tensor.matmul(
                        wps[:, 0:64],
                        warm_src[:, 0:128],
                        warm_src[:, 0:64],
                        start=True,
                        stop=True,
                    )

            warm_mm(150)

            # ---- V projection ----
            # First half (st 0-7): c-outer / st-inner over 8 PSUM banks so
            # matmuls for chunk c start as soon as vT[:, c, :] lands.
            # Second half (st 8-15): all data is resident by then, so go
            # st-outer — one bank at a time — which spreads the DVE drain
            # copies instead of bunching all 8 at the end (that bunching
            # stalled the next PSUM allocation for ~6us).
            # bv is added during the DVE copy.
            vps = [
                ps_s1.tile([128, 512], F32, tag="ps", name=f"vpsA{i}")
                for i in range(8)
            ]
            for c in range(NDM):
                for i in range(8):
                    nc.tensor.matmul(
                        vps[i][:],
                        acts["v"][:, c, i * 128 : (i + 1) * 128],
                        wts["v"][:, c, :],
                        start=(c == 0),
                        stop=(c == NDM - 1),
                    )

            def vp_drain(st, ps):
                nc.vector.tensor_tensor(
                    Vp[:, st, :, 0:DV],
                    ps[:].rearrange("p (h d) -> p h d", h=HL),
                    bvb[:],
                    op=ALU.add,
                )

            for i in range(8):
                vp_drain(i, vps[i])
            for i in range(8):
                st = 8 + i
                psB = ps_s1.tile([128, 512], F32, tag="ps", name=f"vpsB{i}")
                for c in range(NDM):
                    nc.tensor.matmul(
                        psB[:],
                        acts["v"][:, c, st * 128 : (st + 1) * 128],
                        wts["v"][:, c, :],
                        start=(c == 0),
                        stop=(c == NDM - 1),
                    )
                vp_drain(st, psB)

            actv_pool.release()

            # ---- Q/K projections for t=0 (quarter-granular) ----
            biases = {"q": bq_sb, "k": bk_sb}
            dsts = {"q": QT, "k": KT}
            for key in ("q", "k"):
                for quarter in range(NQ):
                    qs = slice(quarter * 512, quarter * 512 + 512)
                    ps = ps_s1.tile([128, 512], F32, tag="ps")
                    for c in range(NDM):
                        nc.tensor.matmul(
                            ps[:],
                            wts[key][:, c, 0:128],
                            acts[key][:, quarter, c, :],
                            start=(c == 0),
                            stop=(c == NDM - 1),
                        )
                    nc.vector.tensor_scalar_add(
                        dsts[key][:, 0, qs], ps[:], biases[key][:, 0:1]
                    )
            ps_s1.release()

            # ---- Stage 2: attention with interleaved projections ----
            with tc.tile_pool(name="att", bufs=1) as att_pool:
                AT = att_pool.tile([128, NT, S], BF16)
                with (
                    tc.tile_pool(name="expS", bufs=6) as exp_pool,
                    tc.tile_pool(name="rec", bufs=3) as rec_pool,
                    tc.tile_pool(name="ost2", bufs=3) as ost2_pool,
                    tc.tile_pool(name="ps_sc", bufs=2, space="PSUM") as ps_sc,
                    tc.tile_pool(name="ps_pv", bufs=4, space="PSUM") as ps_pv,
                ):
                    # Flat column stream over (hp, qb, kt) with the PV
                    # matmuls skewed one column behind scores/exp.
                    pv_store = {}
                    proj_fns = {}
                    for hp in range(HL // 2):
                        chains = []
                        if hp < NT - 1:
                            tn = hp + 1
                            # K chains first: KT[tn] must cover full S before
                            # the hp=tn columns start; QT[tn] only needs to
                            # stay ahead of that block's qb loop.
                            for key in ("k", "q"):
                                for quarter in range(NQ):
                                    chains.append((key, tn, quarter))
                        chain_ps = [None]

                        def make_proj_tick(chains, chain_ps):
                            def proj_tick(tick):
                                ci, step = tick // 8, tick % 8
                                if ci >= len(chains):
                                    return
                                key, tn, quarter = chains[ci]
                                if step == 0:
                                    chain_ps[0] = ps_pv.tile(
                                        [128, 512], F32, tag="pv", name="projps"
                                    )
                                cps = chain_ps[0]
                                nc.tensor.matmul(
                                    cps[:],
                                    wts[key][:, step, tn * 128 : (tn + 1) * 128],
                                    acts[key][:, quarter, step, :],
                                    start=(step == 0),
                                    stop=(step == NDM - 1),
                                )
                                if step == NDM - 1:
                                    nc.vector.tensor_scalar_add(
                                        dsts[key][
                                            :,
                                            tn,
                                            quarter * 512 : quarter * 512 + 512,
                                        ],
                                        cps[:],
                                        biases[key][:, tn : tn + 1],
                                    )

                            return proj_tick

                        proj_fns[hp] = make_proj_tick(chains, chain_ps)

                    cols = [
                        (hp, qb, kt)
                        for hp in range(HL // 2)
                        for qb in range(4)
                        for kt in range(NKT)
                    ]

                    def emit_pv(hp, qb, kt, ex):
                        pvs = pv_store[(hp, qb)]
                        for sub in range(2):
                            nc.tensor.matmul(
                                pvs[sub][0 : DV + 1, :],
                                Vp[:, kt, hp * 2 + sub, :],
                                ex[:, sub, :],
                                start=(kt == 0),
                                stop=(kt == NKT - 1),
                            )

                    def emit_norm(hp, qb):
                        t = hp
                        qsl = slice(qb * 512, (qb + 1) * 512)
                        pvs = pv_store.pop((hp, qb))
                        for sub in range(2):
                            psl = slice(sub * 64, sub * 64 + 64)
                            pvp = pvs[sub]
                            rec = rec_pool.tile([1, 512], F32, tag="r")
                            recb = rec_pool.tile([64, 512], F32, tag="rb")
                            dcp = rec_pool.tile([1, 512], F32, tag="d")
                            # custom-DVE ucode mishandles base_partition=64
                            # PSUM reads; stage through partition 0
                            nc.vector.tensor_copy(dcp[:], pvp[DV : DV + 1, :])
                            nc.vector.reciprocal_approx_fast(rec[:], dcp[:])
                            nc.gpsimd.partition_broadcast(recb[:], rec[:])
                            nc.vector.tensor_mul(
                                AT[psl, t, qsl], pvp[0:DV, :], recb[:]
                            )

                    # Output projection fully interleaved into the last
                    # attention columns. Job (m, qh) needs AT[:, :, qh*512:]
                    # complete, i.e. norm(hp=3, qb=qh) at col ~208+16*qh.
                    oproj_jobs = [(m, qh) for qh in range(4) for m in range(NDM)]
                    oproj_state = {"i": 0, "ps": None}

                    def oproj_tick():
                        i = oproj_state["i"]
                        ci, step = i // NT, i % NT
                        if ci >= len(oproj_jobs):
                            return False
                        m, qh = oproj_jobs[ci]
                        hs = slice(qh * 512, qh * 512 + 512)
                        if step == 0:
                            oproj_state["ps"] = ps_pv.tile(
                                [128, 512], F32, tag="pv", name="ops"
                            )
                        ps = oproj_state["ps"]
                        nc.tensor.matmul(
                            ps[:],
                            wo_sb[:, step, m * 128 : (m + 1) * 128],
                            AT[:, step, hs],
                            start=(step == 0),
                            stop=(step == NT - 1),
                        )
                        if step == NT - 1:
                            ot = ost2_pool.tile([128, 512], BF16, tag="os")
                            nc.vector.tensor_copy(ot[:], ps[:])
                            nc.sync.dma_start(
                                outT[m * 128 : (m + 1) * 128, hs], ot[:]
                            )
                        oproj_state["i"] = i + 1
                        return True

                    # PV matmuls are skewed TWO columns behind scores/exp so
                    # the PE never catches the ACT latency tail (a 1-column
                    # skew left ~300-400ns PV-waits-exp stalls at every
                    # group boundary).
                    pipe = []
                    for hp, qb, kt in cols:
                        t = hp
                        qsl = slice(qb * 512, (qb + 1) * 512)
                        if (hp, qb) not in pv_store:
                            pv_store[(hp, qb)] = [
                                ps_pv.tile([128, 512], F32, tag="pv", name=f"pv{i}")
                                for i in range(2)
                            ]
                        scp = ps_sc.tile([128, 2, 512], F32, tag="sc")
                        for sub in range(2):
                            psl = slice(sub * 64, sub * 64 + 64)
                            nc.tensor.matmul(
                                scp[:, sub, :],
                                KT[psl, t, kt * 128 : (kt + 1) * 128],
                                QT[psl, t, qsl],
                                start=True,
                                stop=True,
                            )
                        ex = exp_pool.tile([128, 2, 512], BF16, tag="e")
                        if kt in SCHRAUDOLPH_KT:
                            nc.vector.tensor_scalar(
                                ex[:].bitcast(I16),
                                scp[:],
                                SC_C1,
                                SC_C2,
                                op0=ALU.mult,
                                op1=ALU.add,
                            )
                        else:
                            nc.scalar.activation(ex[:], scp[:], AF.Exp, scale=SCALE)
                        if len(pipe) >= 3:
                            phps, pqb, pkt, pex = pipe.pop(0)
                            emit_pv(phps, pqb, pkt, pex)
                            if pkt == NKT - 1:
                                emit_norm(phps, pqb)
                        proj_fns[hp](qb * NKT + kt)
                        ci_flat = (hp * 4 + qb) * NKT + kt
                        if ci_flat >= 210:
                            oproj_tick()
                            oproj_tick()
                        pipe.append((hp, qb, kt, ex))
                    for phps, pqb, pkt, pex in pipe:
                        emit_pv(phps, pqb, pkt, pex)
                        if pkt == NKT - 1:
                            emit_norm(phps, pqb)
                    # Drain the remaining output-projection jobs in batches
                    # of 4 with steps 0..2 first (they only need AT rows
                    # 0..2, done long ago) so they overlap the final norm's
                    # DVE latency; only the step-3 matmuls wait on it.
                    # Interleaving across jobs also alternates PSUM banks.
                    rem = oproj_state["i"] // NT
                    assert oproj_state["i"] % NT == 0
                    pending = oproj_jobs[rem:]
                    for bstart in range(0, len(pending), 2):
                        batch = pending[bstart : bstart + 2]
                        bps = [
                            ps_pv.tile([128, 512], F32, tag="pv", name="dps")
                            for _ in batch
                        ]
                        for step in range(NT):
                            for (m, qh), ps in zip(batch, bps):
                                hs = slice(qh * 512, qh * 512 + 512)
                                nc.tensor.matmul(
                                    ps[:],
                                    wo_sb[:, step, m * 128 : (m + 1) * 128],
                                    AT[:, step, hs],
                                    start=(step == 0),
                                    stop=(step == NT - 1),
                                )
                        for (m, qh), ps in zip(batch, bps):
                            hs = slice(qh * 512, qh * 512 + 512)
                            ot = ost2_pool.tile([128, 512], BF16, tag="os")
                            nc.vector.tensor_copy(ot[:], ps[:])
                            nc.sync.dma_start(
                                outT[m * 128 : (m + 1) * 128, hs], ot[:]
                            )
            actqk_pool.release()
            wgt_pool.release()

    nc.compile()
    return nc


def get_nc():
    global _CACHED_NC
    if _CACHED_NC is None:
        _CACHED_NC = _build()
    return _CACHED_NC


def _bf(x):
    return np.ascontiguousarray(np.asarray(x, np.float32)).astype(NP_BF16)


def _tile_rows(x):
    # [R, C] -> [128, R//128, C] so each SBUF partition's data is contiguous
    r, c = x.shape
    return np.ascontiguousarray(x.reshape(r // 128, 128, c).transpose(1, 0, 2))


def _quarter_major(x):
    # [128, NDM, S] -> [128, NQ, NDM, 512]: s-quarter-major so early query
    # ranges finish DMA first
    t = x.reshape(128, NDM, NQ, 512)
    return np.ascontiguousarray(t.transpose(0, 2, 1, 3))


def make_in_maps(queries, keys, values, Wq, bq, Wk, bk, Wv, bv, Wo, bo):
    queries = np.asarray(queries, np.float32)
    keys = np.asarray(keys, np.float32)
    values = np.asarray(values, np.float32)
    Wq = np.asarray(Wq, np.float32)
    Wk = np.asarray(Wk, np.float32)
    Wv = np.asarray(Wv, np.float32)
    Wo = np.asarray(Wo, np.float32)
    bq = np.asarray(bq, np.float32)
    bk = np.asarray(bk, np.float32)
    bv = np.asarray(bv, np.float32)
    in_maps = []
    for core in range(8):
        b, hg = divmod(core, 2)
        sl = slice(hg * NB, (hg + 1) * NB)
        in_maps.append(
            {
                "qT": _quarter_major(_tile_rows(_bf(queries[b].T))),
                "kT": _quarter_major(_tile_rows(_bf(keys[b].T))),
                "vT": _tile_rows(_bf(values[b].T)),
                "wq": _tile_rows(_bf(Wq[:, sl])),
                "wk": _tile_rows(_bf(Wk[:, sl])),
                "wv": _tile_rows(_bf(Wv[:, sl])),
                "wo": _tile_rows(_bf(Wo[sl, :])),
                "bq": np.ascontiguousarray(bq[sl]),
                "bk": np.ascontiguousarray(bk[sl]),
                "bvb": np.ascontiguousarray(
                    np.broadcast_to(
                        _bf(bv[sl]).reshape(1, HL, DV), (128, HL, DV)
                    )
                ),
            }
        )
    return in_maps


def assemble(results, bo):
    bo = np.asarray(bo, np.float32)
    out = np.empty((B, S, DM), np.float32)
    for b in range(B):
        acc = np.asarray(results[2 * b]["outT"], np.float32) + np.asarray(
            results[2 * b + 1]["outT"], np.float32
        )
        out[b] = acc.T + bo
    return out


def run(trace=False, **inputs):
    if trace:
        # NTFF profiling shim: this image's antenv lacks axon_hooks.
        import types

        try:
            from antenv import axon_hooks  # noqa: F401
        except ImportError:
            from trn_agent_boot.trn_boot import _ntff_profile_via_ctypes

            mod = types.ModuleType("antenv.axon_hooks")
            _hook = _ntff_profile_via_ctypes("/opt/axon/libaxon_pjrt.so")
            mod.get_axon_ntff_profile_hook = lambda: _hook
            sys.modules["antenv.axon_hooks"] = mod
    nc = get_nc()
    bo = inputs["bo"]
    in_maps = make_in_maps(**inputs)
    res = run_bass_kernel_spmd(nc, in_maps, list(range(8)), trace=trace)
    return assemble(res.results, bo), res


def kernel(**inputs):
    out, _ = run(trace=False, **inputs)
    return out
